# revision 19
# baseline (speedup 1.0000x reference)
"""Trainium2 Bass kernel for NeoMindAttention (sliding-window attention block).

Computes, per the reference:
    q = (x @ Wq + bq), k = (x @ Wk + bk), v = (x @ Wv + bv)   (heads of 128)
    q, k = RoPE(q, k)          (interleaved pairs, theta=10000)
    ctx  = banded softmax attention, centered window of +-256 keys
    out  = ctx @ Wo + bo
    returns (out, k, v)

Sharding: Megatron-style tensor parallel across 8 cores.
    core = b*4 + hg   handles batch b (of 2) and head group hg (4 heads, 512 dims).
Each core gets x[b]^T (bf16) plus its slices of the weights, computes its
partial output projection; the host sums partials and adds bo.

Layout tricks baked in:
  * q/k are computed transposed ([dims, tokens]) so the head dim lands on
    partitions; weight columns are pre-permuted (evens then odds per head)
    so RoPE is a partition-half swap instead of an interleaved shuffle.
  * Scores are computed as scores^T ([keys, queries]); softmax denominator
    comes for free from an appended ones-column on V in the PV matmul.
  * Band mask: with BQ=128 and w2=256, only the first/last 128-key chunks of
    each 5-chunk span need (triangular) masks; boundary clipping removes
    whole chunks exactly.
"""

import math
import os

import numpy as np
import ml_dtypes

import concourse.bass as bass
import concourse.tile as tile
from concourse import bacc, mybir
from concourse.masks import make_identity

F32 = mybir.dt.float32
BF16 = mybir.dt.bfloat16
AX = mybir.AxisListType
ALU = mybir.AluOpType
ACTF = mybir.ActivationFunctionType

# Full-problem constants
B, T, C, H, D = 2, 4096, 2048, 16, 128
HPC = 4               # heads per core
DIMS = HPC * D        # 512
WINDOW = 512
INV_SQRT_D = 1.0 / math.sqrt(D)


def build_program(*, t=T, c=C, hpc=HPC, debug=False, mm_dt=BF16, reps=1,
                  loop_reps=0):
    """Emit the per-core Bass/Tile program (SPMD; all cores run this).

    reps>1 emits the body multiple times (timing harness only).
    loop_reps>0 wraps the body in a hardware For_i loop (timing harness).
    """
    dims = hpc * 128
    NB = t // 128          # query blocks == key chunks
    NT = t // 512          # 512-token tiles
    CK = c // 128          # contraction chunks
    M = hpc

    nc = bacc.Bacc("TRN2", target_bir_lowering=False, debug=debug)

    # ---- I/O ----
    xT = nc.dram_tensor("xT", [c, t], mm_dt, kind="ExternalInput")
    wq = nc.dram_tensor("wq", [c, dims], mm_dt, kind="ExternalInput")
    wk = nc.dram_tensor("wk", [c, dims], mm_dt, kind="ExternalInput")
    wv = nc.dram_tensor("wv", [c, dims], mm_dt, kind="ExternalInput")
    wo = nc.dram_tensor("wo", [dims, c], mm_dt, kind="ExternalInput")
    bqd = nc.dram_tensor("bqd", [128, M], F32, kind="ExternalInput")
    bkd = nc.dram_tensor("bkd", [128, M], F32, kind="ExternalInput")
    bvb = nc.dram_tensor("bvb", [128, dims], F32, kind="ExternalInput")
    csd = nc.dram_tensor("csd", [128, t], F32, kind="ExternalInput")
    ssd = nc.dram_tensor("ssd", [128, t], F32, kind="ExternalInput")
    m0d = nc.dram_tensor("m0d", [128, 128], mm_dt, kind="ExternalInput")
    m4d = nc.dram_tensor("m4d", [128, 128], mm_dt, kind="ExternalInput")

    outp = nc.dram_tensor("outp", [t, c], F32, kind="ExternalOutput")
    ko = nc.dram_tensor("ko", [M, 128, t], F32, kind="ExternalOutput")
    vo = nc.dram_tensor("vo", [t, dims], F32, kind="ExternalOutput")

    with tile.TileContext(nc) as tc, __import__("contextlib").ExitStack() as ctx:
        ent = ctx.enter_context

        const = ent(tc.tile_pool(name="const", bufs=1))
        xpool = ent(tc.tile_pool(name="xpool", bufs=CK + 12))
        cspool = ent(tc.tile_pool(name="cspool", bufs=3))
        rtmp = ent(tc.tile_pool(name="rtmp", bufs=4))
        qpool = ent(tc.tile_pool(name="qpool", bufs=2 * M + 2))
        kpool = ent(tc.tile_pool(name="kpool", bufs=2 * M + 2))
        kstage = ent(tc.tile_pool(name="kstage", bufs=3))
        vstage = ent(tc.tile_pool(name="vstage", bufs=3))
        vpool = ent(tc.tile_pool(name="vpool", bufs=9 * M))
        spool = ent(tc.tile_pool(name="spool", bufs=4 * M + 2))
        cbpool = ent(tc.tile_pool(name="cbpool", bufs=4))
        ctpool = ent(tc.tile_pool(name="ctpool", bufs=3 * M))
        rpool = ent(tc.tile_pool(name="rpool", bufs=8))
        ostage = ent(tc.tile_pool(name="ostage", bufs=4))
        pbig = ent(tc.tile_pool(name="pbig", bufs=6, space="PSUM"))
        psm = ent(tc.tile_pool(name="psm", bufs=2, space="PSUM"))

        # ---- constant tiles (DMAs deferred to load_consts) ----
        wq_sb = const.tile([128, CK, dims], mm_dt, name="wq_sb")
        wk_sb = const.tile([128, CK, dims], mm_dt, name="wk_sb")
        wv_sb = const.tile([128, CK, dims], mm_dt, name="wv_sb")
        wo_sb = const.tile([128, M, c], mm_dt, name="wo_sb")
        bq_sb = const.tile([128, M], F32, name="bq_sb")
        bk_sb = const.tile([128, M], F32, name="bk_sb")
        bvb_sb = const.tile([128, dims], F32, name="bvb_sb")
        m0_sb = const.tile([128, 128], mm_dt, name="m0_sb")
        m4_sb = const.tile([128, 128], mm_dt, name="m4_sb")
        ident = const.tile([128, 128], mm_dt, name="ident")

        def load_consts():
            # in (rough) first-use order so the first matmuls start sooner
            nc.sync.dma_start(out=bq_sb, in_=bqd[:, :])
            nc.sync.dma_start(out=bk_sb, in_=bkd[:, :])
            wqr = wq.rearrange("(ck p) m -> p ck m", p=128)
            wkr = wk.rearrange("(ck p) m -> p ck m", p=128)
            for m in range(M):
                sl = slice(m * 128, (m + 1) * 128)
                if m == 0:
                    # finer granularity so the first accumulation chain
                    # can start as soon as its first chunks land
                    q4 = max(1, CK // 4)
                    for g in range(0, CK, q4):
                        gs = slice(g, g + q4)
                        nc.sync.dma_start(
                            out=wq_sb[:, gs, sl], in_=wqr[:, gs, sl]
                        )
                        nc.sync.dma_start(
                            out=wk_sb[:, gs, sl], in_=wkr[:, gs, sl]
                        )
                else:
                    nc.sync.dma_start(out=wq_sb[:, :, sl], in_=wqr[:, :, sl])
                    nc.sync.dma_start(out=wk_sb[:, :, sl], in_=wkr[:, :, sl])
            nc.sync.dma_start(
                out=wv_sb, in_=wv.rearrange("(ck p) m -> p ck m", p=128)
            )
            nc.sync.dma_start(out=bvb_sb, in_=bvb[:, :])
            nc.sync.dma_start(out=m0_sb, in_=m0d[:, :])
            nc.sync.dma_start(out=m4_sb, in_=m4d[:, :])
            make_identity(nc, ident)
            nc.sync.dma_start(
                out=wo_sb, in_=wo.rearrange("(m p) n -> p m n", p=128)
            )

        QT = {}   # (m, n512) -> [128, 512] bf16 q^T tile
        KT = {}   # (m, n512) -> [128, 512] bf16 k^T tile
        VC = {}   # (m, kc)   -> [128, 129] bf16 v chunk (ones col appended)
        ES = {}   # (m, kc)   -> (strip tile [128, <=640] bf16, nlo)
        CT = {}   # (m, blk)  -> [128, 128] bf16 normalized ctx^T
        state = (QT, KT, VC, ES, CT)

        def rope_evict(psum, bias_col, cs_t, ss_t, out_bf, out_f32=None):
            # sw = swap-halves(psum + bias);  t0 = (psum + bias) * cos
            sw = rtmp.tile([128, 512], F32, name="sw", tag="sw")
            nc.vector.tensor_scalar(
                out=sw[0:64, :], in0=psum[64:128, :],
                scalar1=bias_col[64:128, :], scalar2=None, op0=ALU.add,
            )
            nc.vector.tensor_scalar(
                out=sw[64:128, :], in0=psum[0:64, :],
                scalar1=bias_col[0:64, :], scalar2=None, op0=ALU.add,
            )
            t0 = rtmp.tile([128, 512], F32, name="t0", tag="t0")
            nc.vector.scalar_tensor_tensor(
                out=t0, in0=psum, scalar=bias_col, in1=cs_t,
                op0=ALU.add, op1=ALU.mult,
            )
            nc.vector.tensor_mul(out=sw, in0=sw, in1=ss_t)
            if out_f32 is not None:
                nc.vector.tensor_add(out=out_f32, in0=t0, in1=sw)
                nc.scalar.copy(out=out_bf, in_=out_f32)
            else:
                nc.vector.tensor_add(out=out_bf, in0=t0, in1=sw)

        def emit_strip(m, kc):
            """scores^T for key chunk kc vs query blocks kc-2..kc+2."""
            nlo = max(0, kc - 2)
            nhi = min(NB - 1, kc + 2)
            w = (nhi - nlo + 1) * 128
            lhsT = KT[(m, kc // 4)][:, (kc % 4) * 128:(kc % 4) * 128 + 128]
            sp = pbig.tile([128, 512], F32, name="sp", tag="pb")
            sp2 = None
            if w > 512:
                sp2 = psm.tile([128, 128], F32, name="sp2", tag="ps")
            # split rhs by source q tile and by the 512-col psum boundary
            col = 0
            nblk = nlo
            while nblk <= nhi:
                n512 = nblk // 4
                ghi = min(nhi, n512 * 4 + 3)          # last block in this q tile
                # also split at psum boundary (col 512)
                npc = ghi - nblk + 1
                if col < 512 and col + npc * 128 > 512:
                    npc = (512 - col) // 128
                    ghi = nblk + npc - 1
                rhs = QT[(m, n512)][:, (nblk % 4) * 128:(nblk % 4) * 128 + npc * 128]
                if col < 512:
                    dst = sp[:, col:col + npc * 128]
                else:
                    dst = sp2[:, col - 512:col - 512 + npc * 128]
                nc.tensor.matmul(dst, lhsT, rhs, start=True, stop=True)
                col += npc * 128
                nblk = ghi + 1
            st = spool.tile([128, 640], mm_dt, name="st", tag="st")
            nc.scalar.activation(
                out=st[:, 0:min(w, 512)], in_=sp[:, 0:min(w, 512)],
                func=ACTF.Exp, scale=INV_SQRT_D,
            )
            if w > 512:
                nc.scalar.activation(
                    out=st[:, 512:w], in_=sp2[:, 0:w - 512],
                    func=ACTF.Exp, scale=INV_SQRT_D,
                )
            # masks: block kc+2 gets the j0 (l>=t) mask, block kc-2 the j4 (l<=t)
            if kc + 2 <= NB - 1:
                o = (kc + 2 - nlo) * 128
                nc.vector.tensor_mul(
                    out=st[:, o:o + 128], in0=st[:, o:o + 128], in1=m0_sb
                )
            if kc - 2 >= 0:
                o = (kc - 2 - nlo) * 128
                nc.vector.tensor_mul(
                    out=st[:, o:o + 128], in0=st[:, o:o + 128], in1=m4_sb
                )
            ES[(m, kc)] = (st, nlo)

        def emit_pv(m, blk):
            klo = max(0, blk - 2)
            khi = min(NB - 1, blk + 2)
            cp = pbig.tile([128, 512], F32, name="cp", tag="pb")
            for i, kc in enumerate(range(klo, khi + 1)):
                st, nlo = ES[(m, kc)]
                o = (blk - nlo) * 128
                nc.tensor.matmul(
                    cp[:, 0:129], st[:, o:o + 128], VC[(m, kc)],
                    start=(kc == klo), stop=(kc == khi),
                )
            rc = rpool.tile([128, 1], F32, name="rc", tag="rc")
            nc.vector.reciprocal(out=rc, in_=cp[:, 128:129])
            cb = cbpool.tile([128, 128], mm_dt, name="cb", tag="cb")
            nc.scalar.activation(
                out=cb, in_=cp[:, 0:128], func=ACTF.Copy, scale=rc
            )
            tp = psm.tile([128, 128], mm_dt, name="tp", tag="ps")
            nc.tensor.transpose(tp, cb, ident)
            ct = ctpool.tile([128, 128], mm_dt, name="ct", tag="ct")
            nc.vector.tensor_copy(out=ct, in_=tp)
            CT[(m, blk)] = ct

        def emit_outproj(blk):
            for nt4 in range(c // 512):
                op = pbig.tile([128, 512], F32, name="op", tag="pb")
                for m in range(M):
                    nc.tensor.matmul(
                        op, CT[(m, blk)], wo_sb[:, m, nt4 * 512:(nt4 + 1) * 512],
                        start=(m == 0), stop=(m == M - 1),
                    )
                os_ = ostage.tile([128, 512], F32, name="os_", tag="os")
                nc.vector.tensor_copy(out=os_, in_=op)
                nc.sync.dma_start(
                    out=outp[blk * 128:(blk + 1) * 128, nt4 * 512:(nt4 + 1) * 512],
                    in_=os_,
                )
            for m in range(M):
                del CT[(m, blk)]

        def strips_for(n512):
            lo = max(0, 4 * n512 - 2)
            hi = 4 * n512 + 2 if n512 < NT - 1 else NB
            return range(lo, hi)

        import contextlib

        def body(load=False):
          for d in state:
            d.clear()
          for n512 in range(NT):
            tok = n512 * 512
            xc = []
            for ck in range(CK):
                xt_ = xpool.tile([128, 512], mm_dt, name="xt_", tag="xt")
                nc.sync.dma_start(
                    out=xt_, in_=xT[ck * 128:(ck + 1) * 128, tok:tok + 512]
                )
                xc.append(xt_)
            cs_t = cspool.tile([128, 512], F32, name="cs_t", tag="cs")
            nc.sync.dma_start(out=cs_t, in_=csd[:, tok:tok + 512])
            ss_t = cspool.tile([128, 512], F32, name="ss_t", tag="ss")
            nc.sync.dma_start(out=ss_t, in_=ssd[:, tok:tok + 512])
            if load and n512 == 0:
                load_consts()

            for m in range(M):
                qp = pbig.tile([128, 512], F32, name="qp", tag="pb")
                for ck in range(CK):
                    nc.tensor.matmul(
                        qp, wq_sb[:, ck, m * 128:(m + 1) * 128], xc[ck],
                        start=(ck == 0), stop=(ck == CK - 1),
                    )
                qt = qpool.tile([128, 512], mm_dt, name="qt", tag="qt")
                rope_evict(qp, bq_sb[:, m:m + 1], cs_t, ss_t, qt)
                QT[(m, n512)] = qt

                kp = pbig.tile([128, 512], F32, name="kp", tag="pb")
                for ck in range(CK):
                    nc.tensor.matmul(
                        kp, wk_sb[:, ck, m * 128:(m + 1) * 128], xc[ck],
                        start=(ck == 0), stop=(ck == CK - 1),
                    )
                ks = kstage.tile([128, 512], F32, name="ks", tag="ks")
                kt = kpool.tile([128, 512], mm_dt, name="kt", tag="kt")
                rope_evict(kp, bk_sb[:, m:m + 1], cs_t, ss_t, kt, out_f32=ks)
                nc.sync.dma_start(out=ko[m, :, tok:tok + 512], in_=ks)
                KT[(m, n512)] = kt

            for t4 in range(4):
                kc = n512 * 4 + t4
                vp = pbig.tile([128, 512], F32, name="vp", tag="pb")
                for ck in range(CK):
                    nc.tensor.matmul(
                        vp[:, 0:dims], xc[ck][:, t4 * 128:(t4 + 1) * 128],
                        wv_sb[:, ck, :],
                        start=(ck == 0), stop=(ck == CK - 1),
                    )
                vs = vstage.tile([128, dims], F32, name="vs", tag="vs")
                nc.vector.tensor_add(out=vs, in0=vp[:, 0:dims], in1=bvb_sb)
                nc.sync.dma_start(out=vo[kc * 128:(kc + 1) * 128, :], in_=vs)
                for m in range(M):
                    vt = vpool.tile([128, 129], mm_dt, name="vt", tag="vt")
                    nc.vector.tensor_copy(
                        out=vt[:, 0:128], in_=vs[:, m * 128:(m + 1) * 128]
                    )
                    nc.vector.memset(vt[:, 128:129], 1.0)
                    VC[(m, kc)] = vt

            for kc in strips_for(n512):
                for m in range(M):
                    emit_strip(m, kc)
                    if kc - 2 >= 0:
                        emit_pv(m, kc - 2)
                if kc - 2 >= 0:
                    emit_outproj(kc - 2)
          # flush: blocks NB-2, NB-1 (virtual strips NB, NB+1)
          for kc in (NB, NB + 1):
            for m in range(M):
                emit_pv(m, kc - 2)
            emit_outproj(kc - 2)

        if loop_reps > 0:
            load_consts()
            with tc.For_i(0, loop_reps, 1):
                body()
        else:
            for _rep in range(reps):
                body(load=(_rep == 0))

    nc.compile()
    return nc


_PROGRAM = None


def _get_program():
    global _PROGRAM
    if _PROGRAM is None:
        _PROGRAM = build_program()
    return _PROGRAM


def _host_inputs(x, Wq, bq, Wk, bk, Wv, bv, Wo):
    """Per-core input maps (8 cores: core = b*4 + hg)."""
    bf16 = ml_dtypes.bfloat16
    perm = np.concatenate([np.arange(0, 128, 2), np.arange(1, 128, 2)])

    theta = (
        1.0 / (10000.0 ** (np.arange(0, D, 2, dtype=np.float32) / np.float32(D)))
    ).astype(np.float32)
    pos = np.arange(T, dtype=np.float32)
    freqs = np.einsum("i,j->ij", pos, theta).astype(np.float32)  # [T, 64]
    cosv = np.cos(freqs).astype(np.float32).T                    # [64, T]
    sinv = np.sin(freqs).astype(np.float32).T
    cs = np.ascontiguousarray(np.concatenate([cosv, cosv], axis=0))
    ss = np.ascontiguousarray(np.concatenate([-sinv, sinv], axis=0))

    li = np.arange(128)
    m0 = (li[:, None] >= li[None, :]).astype(bf16)
    m4 = (li[:, None] <= li[None, :]).astype(bf16)

    xT_b = {}
    for b in range(B):
        xT_b[b] = np.ascontiguousarray(x[b].T).astype(bf16)

    in_maps = []
    for core in range(8):
        b, hg = core // 4, core % 4
        base = hg * DIMS
        pcols = base + np.concatenate([h * 128 + perm for h in range(HPC)])
        ncols = base + np.arange(DIMS)
        in_maps.append(
            dict(
                xT=xT_b[b],
                wq=np.ascontiguousarray(Wq[:, pcols]).astype(bf16),
                wk=np.ascontiguousarray(Wk[:, pcols]).astype(bf16),
                wv=np.ascontiguousarray(Wv[:, ncols]).astype(bf16),
                wo=np.ascontiguousarray(Wo[ncols, :]).astype(bf16),
                bqd=np.ascontiguousarray(
                    bq[pcols].reshape(HPC, 128).T
                ).astype(np.float32),
                bkd=np.ascontiguousarray(
                    bk[pcols].reshape(HPC, 128).T
                ).astype(np.float32),
                bvb=np.ascontiguousarray(
                    np.broadcast_to(bv[ncols], (128, DIMS))
                ).astype(np.float32),
                csd=cs,
                ssd=ss,
                m0d=m0,
                m4d=m4,
            )
        )
    return in_maps, perm


def _gather(results, bo):
    perm = np.concatenate([np.arange(0, 128, 2), np.arange(1, 128, 2)])
    out = np.zeros((B, T, C), np.float32)
    k = np.empty((B, H, T, D), np.float32)
    v = np.empty((B, H, T, D), np.float32)
    for core in range(8):
        b, hg = core // 4, core % 4
        r = results[core]
        out[b] += r["outp"]
        for h in range(HPC):
            karr = np.empty((T, D), np.float32)
            karr[:, perm] = r["ko"][h].T
            k[b, hg * HPC + h] = karr
            v[b, hg * HPC + h] = r["vo"][:, h * 128:(h + 1) * 128]
    out += bo.astype(np.float32)
    return out, k, v


def kernel(x, Wq, bq, Wk, bk, Wv, bv, Wo, bo):
    from concourse.bass_utils import run_bass_kernel_spmd

    x = np.asarray(x)
    nc = _get_program()
    in_maps, _ = _host_inputs(
        x, np.asarray(Wq), np.asarray(bq), np.asarray(Wk), np.asarray(bk),
        np.asarray(Wv), np.asarray(bv), np.asarray(Wo)
    )
    res = run_bass_kernel_spmd(nc, in_maps, list(range(8)))
    return _gather(res.results, np.asarray(bo))


# revision 36
# speedup vs baseline: 1.3004x; 1.3004x over previous
"""Trainium2 Bass kernel for NeoMindAttention (sliding-window attention block).

Computes, per the reference:
    q = (x @ Wq + bq), k = (x @ Wk + bk), v = (x @ Wv + bv)   (heads of 128)
    q, k = RoPE(q, k)          (interleaved pairs, theta=10000)
    ctx  = banded softmax attention, centered window of +-256 keys
    out  = ctx @ Wo + bo
    returns (out, k, v)

Sharding: Megatron-style tensor parallel across 8 cores.
    core = b*4 + hg   handles batch b (of 2) and head group hg (4 heads, 512 dims).
Each core gets x[b]^T (bf16) plus its slices of the weights, computes its
partial output projection; the host sums partials and adds bo.

Layout tricks baked in:
  * q/k are computed transposed ([dims, tokens]) so the head dim lands on
    partitions; weight columns are pre-permuted (evens then odds per head)
    so RoPE is a partition-half swap instead of an interleaved shuffle.
  * Scores are computed as scores^T ([keys, queries]); softmax denominator
    comes for free from an appended ones-column on V in the PV matmul.
  * Band mask: with BQ=128 and w2=256, only the first/last 128-key chunks of
    each 5-chunk span need (triangular) masks; boundary clipping removes
    whole chunks exactly.
"""

import math
import os

import numpy as np
import ml_dtypes

import concourse.bass as bass
import concourse.tile as tile
from concourse import bacc, mybir
from concourse.masks import make_identity

F32 = mybir.dt.float32
BF16 = mybir.dt.bfloat16
AX = mybir.AxisListType
ALU = mybir.AluOpType
ACTF = mybir.ActivationFunctionType

# Full-problem constants
B, T, C, H, D = 2, 4096, 2048, 16, 128
HPC = 4               # heads per core
DIMS = HPC * D        # 512
WINDOW = 512
INV_SQRT_D = 1.0 / math.sqrt(D)


def build_program(*, t=T, c=C, hpc=HPC, debug=False, mm_dt=BF16, reps=1,
                  loop_reps=0):
    """Emit the per-core Bass/Tile program (SPMD; all cores run this).

    reps>1 emits the body multiple times (timing harness only).
    loop_reps>0 wraps the body in a hardware For_i loop (timing harness).
    """
    dims = hpc * 128
    NB = t // 128          # query blocks == key chunks
    NT = t // 512          # 512-token tiles
    CK = c // 128          # contraction chunks
    M = hpc

    nc = bacc.Bacc("TRN2", target_bir_lowering=False, debug=debug)

    # ---- I/O ----
    xT = nc.dram_tensor("xT", [c, t], mm_dt, kind="ExternalInput")
    wq = nc.dram_tensor("wq", [c, dims], mm_dt, kind="ExternalInput")
    wk = nc.dram_tensor("wk", [c, dims], mm_dt, kind="ExternalInput")
    wv = nc.dram_tensor("wv", [c, dims], mm_dt, kind="ExternalInput")
    wo = nc.dram_tensor("wo", [dims, c], mm_dt, kind="ExternalInput")
    bqd = nc.dram_tensor("bqd", [128, M], F32, kind="ExternalInput")
    bkd = nc.dram_tensor("bkd", [128, M], F32, kind="ExternalInput")
    bvb = nc.dram_tensor("bvb", [128, dims], F32, kind="ExternalInput")
    csd = nc.dram_tensor("csd", [128, t], F32, kind="ExternalInput")
    ssd = nc.dram_tensor("ssd", [128, t], F32, kind="ExternalInput")
    m0d = nc.dram_tensor("m0d", [128, 128], mm_dt, kind="ExternalInput")
    m4d = nc.dram_tensor("m4d", [128, 128], mm_dt, kind="ExternalInput")

    outp = nc.dram_tensor("outp", [t, c], F32, kind="ExternalOutput")
    ko = nc.dram_tensor("ko", [M, 128, t], F32, kind="ExternalOutput")
    vo = nc.dram_tensor("vo", [t, dims], F32, kind="ExternalOutput")

    with tile.TileContext(nc) as tc, __import__("contextlib").ExitStack() as ctx:
        ent = ctx.enter_context

        const = ent(tc.tile_pool(name="const", bufs=1))
        xpool = ent(tc.tile_pool(name="xpool", bufs=CK + 12))
        cspool = ent(tc.tile_pool(name="cspool", bufs=3))
        rtmp = ent(tc.tile_pool(name="rtmp", bufs=4))
        qpool = ent(tc.tile_pool(name="qpool", bufs=2 * M + 2))
        kpool = ent(tc.tile_pool(name="kpool", bufs=2 * M + 2))
        kstage = ent(tc.tile_pool(name="kstage", bufs=3))
        vstage = ent(tc.tile_pool(name="vstage", bufs=3))
        vpool = ent(tc.tile_pool(name="vpool", bufs=9 * M))
        spool = ent(tc.tile_pool(name="spool", bufs=4 * M + 2))
        cbpool = ent(tc.tile_pool(name="cbpool", bufs=4))
        ctpool = ent(tc.tile_pool(name="ctpool", bufs=3 * M))
        rpool = ent(tc.tile_pool(name="rpool", bufs=8))
        ostage = ent(tc.tile_pool(name="ostage", bufs=4))
        pbig = ent(tc.tile_pool(name="pbig", bufs=6, space="PSUM"))
        psm = ent(tc.tile_pool(name="psm", bufs=2, space="PSUM"))

        # ---- constant tiles (DMAs deferred to load_consts) ----
        wq_sb = const.tile([128, CK, dims], mm_dt, name="wq_sb")
        wk_sb = const.tile([128, CK, dims], mm_dt, name="wk_sb")
        wv_sb = const.tile([128, CK, dims], mm_dt, name="wv_sb")
        wo_sb = const.tile([128, M, c], mm_dt, name="wo_sb")
        bq_sb = const.tile([128, M], F32, name="bq_sb")
        bk_sb = const.tile([128, M], F32, name="bk_sb")
        bvb_sb = const.tile([128, dims], F32, name="bvb_sb")
        m0_sb = const.tile([128, 128], mm_dt, name="m0_sb")
        m4_sb = const.tile([128, 128], mm_dt, name="m4_sb")
        ident = const.tile([128, 128], mm_dt, name="ident")

        def load_consts():
            # in (rough) first-use order so the first matmuls start sooner
            nc.sync.dma_start(out=bq_sb, in_=bqd[:, :])
            nc.sync.dma_start(out=bk_sb, in_=bkd[:, :])
            wqr = wq.rearrange("(ck p) m -> p ck m", p=128)
            wkr = wk.rearrange("(ck p) m -> p ck m", p=128)
            for m in range(M):
                sl = slice(m * 128, (m + 1) * 128)
                if m == 0:
                    # finer granularity so the first accumulation chain
                    # can start as soon as its first chunks land
                    q4 = max(1, CK // 4)
                    for g in range(0, CK, q4):
                        gs = slice(g, g + q4)
                        nc.sync.dma_start(
                            out=wq_sb[:, gs, sl], in_=wqr[:, gs, sl]
                        )
                        nc.sync.dma_start(
                            out=wk_sb[:, gs, sl], in_=wkr[:, gs, sl]
                        )
                else:
                    nc.sync.dma_start(out=wq_sb[:, :, sl], in_=wqr[:, :, sl])
                    nc.sync.dma_start(out=wk_sb[:, :, sl], in_=wkr[:, :, sl])
            nc.sync.dma_start(
                out=wv_sb, in_=wv.rearrange("(ck p) m -> p ck m", p=128)
            )
            nc.sync.dma_start(out=bvb_sb, in_=bvb[:, :])
            nc.sync.dma_start(out=m0_sb, in_=m0d[:, :])
            nc.sync.dma_start(out=m4_sb, in_=m4d[:, :])
            make_identity(nc, ident)
            nc.sync.dma_start(
                out=wo_sb, in_=wo.rearrange("(m p) n -> p m n", p=128)
            )

        QT = {}   # (m, n512) -> [128, 512] bf16 q^T tile
        KT = {}   # (m, n512) -> [128, 512] bf16 k^T tile
        VC = {}   # (m, kc)   -> [128, 129] bf16 v chunk (ones col appended)
        ES = {}   # (m, kc)   -> (strip tile [128, <=640] bf16, nlo)
        CT = {}   # (m, blk)  -> [128, 128] bf16 normalized ctx^T
        state = (QT, KT, VC, ES, CT)

        def rope_evict(psum, bias_col, cs_t, ss_t, out_bf, out_f32=None):
            # sw = swap-halves(psum + bias);  t0 = (psum + bias) * cos
            sw = rtmp.tile([128, 512], F32, name="sw", tag="sw")
            nc.vector.tensor_scalar(
                out=sw[0:64, :], in0=psum[64:128, :],
                scalar1=bias_col[64:128, :], scalar2=None, op0=ALU.add,
            )
            nc.vector.tensor_scalar(
                out=sw[64:128, :], in0=psum[0:64, :],
                scalar1=bias_col[0:64, :], scalar2=None, op0=ALU.add,
            )
            t0 = rtmp.tile([128, 512], F32, name="t0", tag="t0")
            nc.vector.scalar_tensor_tensor(
                out=t0, in0=psum, scalar=bias_col, in1=cs_t,
                op0=ALU.add, op1=ALU.mult,
            )
            nc.vector.tensor_mul(out=sw, in0=sw, in1=ss_t)
            if out_f32 is not None:
                nc.vector.tensor_add(out=out_f32, in0=t0, in1=sw)
                nc.scalar.copy(out=out_bf, in_=out_f32)
            else:
                nc.vector.tensor_add(out=out_bf, in0=t0, in1=sw)

        def emit_strip(m, kc):
            """scores^T for key chunk kc vs query blocks kc-2..kc+2."""
            nlo = max(0, kc - 2)
            nhi = min(NB - 1, kc + 2)
            w = (nhi - nlo + 1) * 128
            lhsT = KT[(m, kc // 4)][:, (kc % 4) * 128:(kc % 4) * 128 + 128]
            sp = pbig.tile([128, 512], F32, name="sp", tag="pb")
            sp2 = None
            if w > 512:
                sp2 = psm.tile([128, 128], F32, name="sp2", tag="ps")
            # split rhs by source q tile and by the 512-col psum boundary
            col = 0
            nblk = nlo
            while nblk <= nhi:
                n512 = nblk // 4
                ghi = min(nhi, n512 * 4 + 3)          # last block in this q tile
                # also split at psum boundary (col 512)
                npc = ghi - nblk + 1
                if col < 512 and col + npc * 128 > 512:
                    npc = (512 - col) // 128
                    ghi = nblk + npc - 1
                rhs = QT[(m, n512)][:, (nblk % 4) * 128:(nblk % 4) * 128 + npc * 128]
                if col < 512:
                    dst = sp[:, col:col + npc * 128]
                else:
                    dst = sp2[:, col - 512:col - 512 + npc * 128]
                nc.tensor.matmul(dst, lhsT, rhs, start=True, stop=True)
                col += npc * 128
                nblk = ghi + 1
            st = spool.tile([128, 640], mm_dt, name="st", tag="st")
            nc.scalar.activation(
                out=st[:, 0:min(w, 512)], in_=sp[:, 0:min(w, 512)],
                func=ACTF.Exp, scale=INV_SQRT_D,
            )
            if w > 512:
                nc.scalar.activation(
                    out=st[:, 512:w], in_=sp2[:, 0:w - 512],
                    func=ACTF.Exp, scale=INV_SQRT_D,
                )
            # masks: block kc+2 gets the j0 (l>=t) mask, block kc-2 the j4 (l<=t)
            if kc + 2 <= NB - 1:
                o = (kc + 2 - nlo) * 128
                nc.vector.tensor_mul(
                    out=st[:, o:o + 128], in0=st[:, o:o + 128], in1=m0_sb
                )
            if kc - 2 >= 0:
                o = (kc - 2 - nlo) * 128
                nc.vector.tensor_mul(
                    out=st[:, o:o + 128], in0=st[:, o:o + 128], in1=m4_sb
                )
            ES[(m, kc)] = (st, nlo)

        def emit_pv(m, blk):
            klo = max(0, blk - 2)
            khi = min(NB - 1, blk + 2)
            cp = pbig.tile([128, 512], F32, name="cp", tag="pb")
            for i, kc in enumerate(range(klo, khi + 1)):
                st, nlo = ES[(m, kc)]
                o = (blk - nlo) * 128
                nc.tensor.matmul(
                    cp[:, 0:129], st[:, o:o + 128], VC[(m, kc)],
                    start=(kc == klo), stop=(kc == khi),
                )
            rc = rpool.tile([128, 1], F32, name="rc", tag="rc")
            nc.vector.reciprocal(out=rc, in_=cp[:, 128:129])
            cb = cbpool.tile([128, 128], mm_dt, name="cb", tag="cb")
            nc.vector.tensor_scalar_mul(out=cb, in0=cp[:, 0:128], scalar1=rc)
            tp = psm.tile([128, 128], mm_dt, name="tp", tag="ps")
            nc.tensor.transpose(tp, cb, ident)
            ct = ctpool.tile([128, 128], mm_dt, name="ct", tag="ct")
            nc.vector.tensor_copy(out=ct, in_=tp)
            CT[(m, blk)] = ct

        def emit_outproj(blk):
            for nt4 in range(c // 512):
                op = pbig.tile([128, 512], F32, name="op", tag="pb")
                for m in range(M):
                    nc.tensor.matmul(
                        op, CT[(m, blk)], wo_sb[:, m, nt4 * 512:(nt4 + 1) * 512],
                        start=(m == 0), stop=(m == M - 1),
                    )
                os_ = ostage.tile([128, 512], F32, name="os_", tag="os")
                nc.vector.tensor_copy(out=os_, in_=op)
                nc.sync.dma_start(
                    out=outp[blk * 128:(blk + 1) * 128, nt4 * 512:(nt4 + 1) * 512],
                    in_=os_,
                )
            for m in range(M):
                del CT[(m, blk)]

        def strips_for(n512):
            lo = max(0, 4 * n512 - 2)
            hi = 4 * n512 + 2 if n512 < NT - 1 else NB
            return range(lo, hi)

        import contextlib

        def body(load=False):
          for d in state:
            d.clear()
          for n512 in range(NT):
            tok = n512 * 512
            xc = []
            for ck in range(CK):
                xt_ = xpool.tile([128, 512], mm_dt, name="xt_", tag="xt")
                eng = nc.sync if ck % 2 == 0 else nc.gpsimd
                eng.dma_start(
                    out=xt_, in_=xT[ck * 128:(ck + 1) * 128, tok:tok + 512]
                )
                xc.append(xt_)
            cs_t = cspool.tile([128, 512], F32, name="cs_t", tag="cs")
            nc.sync.dma_start(out=cs_t, in_=csd[:, tok:tok + 512])
            ss_t = cspool.tile([128, 512], F32, name="ss_t", tag="ss")
            nc.sync.dma_start(out=ss_t, in_=ssd[:, tok:tok + 512])
            if load and n512 == 0:
                load_consts()

            for m in range(M):
                qp = pbig.tile([128, 512], F32, name="qp", tag="pb")
                for ck in range(CK):
                    nc.tensor.matmul(
                        qp, wq_sb[:, ck, m * 128:(m + 1) * 128], xc[ck],
                        start=(ck == 0), stop=(ck == CK - 1),
                    )
                qt = qpool.tile([128, 512], mm_dt, name="qt", tag="qt")
                rope_evict(qp, bq_sb[:, m:m + 1], cs_t, ss_t, qt)
                QT[(m, n512)] = qt

                kp = pbig.tile([128, 512], F32, name="kp", tag="pb")
                for ck in range(CK):
                    nc.tensor.matmul(
                        kp, wk_sb[:, ck, m * 128:(m + 1) * 128], xc[ck],
                        start=(ck == 0), stop=(ck == CK - 1),
                    )
                ks = kstage.tile([128, 512], F32, name="ks", tag="ks")
                kt = kpool.tile([128, 512], mm_dt, name="kt", tag="kt")
                rope_evict(kp, bk_sb[:, m:m + 1], cs_t, ss_t, kt, out_f32=ks)
                nc.sync.dma_start(out=ko[m, :, tok:tok + 512], in_=ks)
                KT[(m, n512)] = kt

            for t4 in range(4):
                kc = n512 * 4 + t4
                vp = pbig.tile([128, 512], F32, name="vp", tag="pb")
                for ck in range(CK):
                    nc.tensor.matmul(
                        vp[:, 0:dims], xc[ck][:, t4 * 128:(t4 + 1) * 128],
                        wv_sb[:, ck, :],
                        start=(ck == 0), stop=(ck == CK - 1),
                    )
                vs = vstage.tile([128, dims], F32, name="vs", tag="vs")
                nc.vector.tensor_add(out=vs, in0=vp[:, 0:dims], in1=bvb_sb)
                nc.sync.dma_start(out=vo[kc * 128:(kc + 1) * 128, :], in_=vs)
                for m in range(M):
                    vt = vpool.tile([128, 129], mm_dt, name="vt", tag="vt")
                    nc.vector.tensor_copy(
                        out=vt[:, 0:128], in_=vs[:, m * 128:(m + 1) * 128]
                    )
                    nc.vector.memset(vt[:, 128:129], 1.0)
                    VC[(m, kc)] = vt

            for kc in strips_for(n512):
                for m in range(M):
                    emit_strip(m, kc)
                    if kc - 2 >= 0:
                        emit_pv(m, kc - 2)
                if kc - 2 >= 0:
                    emit_outproj(kc - 2)
          # flush: blocks NB-2, NB-1 (virtual strips NB, NB+1)
          for kc in (NB, NB + 1):
            for m in range(M):
                emit_pv(m, kc - 2)
            emit_outproj(kc - 2)

        if loop_reps > 0:
            load_consts()
            with tc.For_i(0, loop_reps, 1):
                body()
        else:
            for _rep in range(reps):
                body(load=(_rep == 0))

    nc.compile()
    return nc


_PROGRAM = None


def _get_program():
    global _PROGRAM
    if _PROGRAM is None:
        _PROGRAM = build_program()
    return _PROGRAM


def _host_inputs(x, Wq, bq, Wk, bk, Wv, bv, Wo):
    """Per-core input maps (8 cores: core = b*4 + hg)."""
    bf16 = ml_dtypes.bfloat16
    perm = np.concatenate([np.arange(0, 128, 2), np.arange(1, 128, 2)])

    theta = (
        1.0 / (10000.0 ** (np.arange(0, D, 2, dtype=np.float32) / np.float32(D)))
    ).astype(np.float32)
    pos = np.arange(T, dtype=np.float32)
    freqs = np.einsum("i,j->ij", pos, theta).astype(np.float32)  # [T, 64]
    cosv = np.cos(freqs).astype(np.float32).T                    # [64, T]
    sinv = np.sin(freqs).astype(np.float32).T
    cs = np.ascontiguousarray(np.concatenate([cosv, cosv], axis=0))
    ss = np.ascontiguousarray(np.concatenate([-sinv, sinv], axis=0))

    li = np.arange(128)
    m0 = (li[:, None] >= li[None, :]).astype(bf16)
    m4 = (li[:, None] <= li[None, :]).astype(bf16)

    xT_b = {}
    for b in range(B):
        xT_b[b] = np.ascontiguousarray(x[b].T).astype(bf16)

    in_maps = []
    for core in range(8):
        b, hg = core // 4, core % 4
        base = hg * DIMS
        pcols = base + np.concatenate([h * 128 + perm for h in range(HPC)])
        ncols = base + np.arange(DIMS)
        in_maps.append(
            dict(
                xT=xT_b[b],
                wq=np.ascontiguousarray(Wq[:, pcols]).astype(bf16),
                wk=np.ascontiguousarray(Wk[:, pcols]).astype(bf16),
                wv=np.ascontiguousarray(Wv[:, ncols]).astype(bf16),
                wo=np.ascontiguousarray(Wo[ncols, :]).astype(bf16),
                bqd=np.ascontiguousarray(
                    bq[pcols].reshape(HPC, 128).T
                ).astype(np.float32),
                bkd=np.ascontiguousarray(
                    bk[pcols].reshape(HPC, 128).T
                ).astype(np.float32),
                bvb=np.ascontiguousarray(
                    np.broadcast_to(bv[ncols], (128, DIMS))
                ).astype(np.float32),
                csd=cs,
                ssd=ss,
                m0d=m0,
                m4d=m4,
            )
        )
    return in_maps, perm


def _gather(results, bo):
    perm = np.concatenate([np.arange(0, 128, 2), np.arange(1, 128, 2)])
    out = np.zeros((B, T, C), np.float32)
    k = np.empty((B, H, T, D), np.float32)
    v = np.empty((B, H, T, D), np.float32)
    for core in range(8):
        b, hg = core // 4, core % 4
        r = results[core]
        out[b] += r["outp"]
        for h in range(HPC):
            karr = np.empty((T, D), np.float32)
            karr[:, perm] = r["ko"][h].T
            k[b, hg * HPC + h] = karr
            v[b, hg * HPC + h] = r["vo"][:, h * 128:(h + 1) * 128]
    out += bo.astype(np.float32)
    return out, k, v


def kernel(x, Wq, bq, Wk, bk, Wv, bv, Wo, bo):
    from concourse.bass_utils import run_bass_kernel_spmd

    x = np.asarray(x)
    nc = _get_program()
    in_maps, _ = _host_inputs(
        x, np.asarray(Wq), np.asarray(bq), np.asarray(Wk), np.asarray(bk),
        np.asarray(Wv), np.asarray(bv), np.asarray(Wo)
    )
    res = run_bass_kernel_spmd(nc, in_maps, list(range(8)))
    return _gather(res.results, np.asarray(bo))
